# revision 10
# baseline (speedup 1.0000x reference)
"""Causal self-attention kernel for 8 trn2 NeuronCores.

Sharding: 2 batch groups x 4 tensor-parallel ranks (Megatron-style head
split).  Core c handles batch b=c//4 and heads [4r, 4r+4) with r=c%4.
Each core:
  1. qk^T projection:   qkT[128h:(128h+128), :] = [q_h^T; k_h^T]  (64+64 rows)
  2. v projection:      v[token, 65h:65h+64], col 65h+64 = 1.0 (den trick)
  3. causal attention in s^T = [key_part, query_free] layout:
       sT = (k^T slice) matmul q^T ; p = exp(s/8) * mask ; y'T += [v|1].T p
     row 64 of y'T is the softmax denominator; normalize via reciprocal +
     partition-broadcast + multiply.
  4. partial out = y_own @ w_out[own head rows, :]  -> [2048, 1024]
  5. ReduceScatter(add) across the 4-rank group: rank r keeps the summed
     rows [512r, 512r+512); add bias -> out [512, 1024]
Host concatenates the 8 x [512, 1024] outputs into [2, 2048, 1024].
"""

import sys

for _p in ("/opt/trn_rl_repo", "/root/.axon_site", "/root/.axon_site/_ro/trn_rl_repo",
           "/root/.axon_site/_ro/pypackages"):
    if _p not in sys.path:
        sys.path.append(_p)

import numpy as np

import concourse.mybir as mybir
import concourse.tile as tile
from concourse import bacc
from concourse import bass_utils

F32 = mybir.dt.float32
BF16 = mybir.dt.bfloat16
F32R = mybir.dt.float32r


def _cfg(B=2, T=2048, C=1024, H=16, n_cores=8, tp=4):
    D = 64
    assert C == H * D
    cfg = dict(B=B, T=T, C=C, H=H, D=D, n_cores=n_cores, tp=tp)
    cfg["groups"] = [[g * tp + r for r in range(tp)] for g in range(n_cores // tp)]
    cfg["HPC"] = H // tp           # heads per core
    cfg["KT"] = C // 128           # contraction tiles for projections
    cfg["NQ"] = T // 512           # 512-wide query chunks
    cfg["TT"] = T // 128           # 128-wide token (key) tiles
    cfg["RT"] = T // tp            # output rows per core
    assert cfg["RT"] % 128 == 0 and T % 512 == 0
    return cfg


CFG = _cfg()


def build_nc(cfg=CFG, dt_mm=BF16, reps=1, no_rs=False):
    B, T, C, H, D = cfg["B"], cfg["T"], cfg["C"], cfg["H"], cfg["D"]
    HPC, KT, NQ, TT, RT = cfg["HPC"], cfg["KT"], cfg["NQ"], cfg["TT"], cfg["RT"]
    tp = cfg["tp"]
    assert HPC % 2 == 0
    Exp = mybir.ActivationFunctionType.Exp

    nc = bacc.Bacc("TRN2", target_bir_lowering=False, debug=False,
                   enable_asserts=True, num_devices=cfg["n_cores"])

    xT = nc.dram_tensor("xT", [C, T], dt_mm, kind="ExternalInput")
    w_qk = nc.dram_tensor("w_qk", [C, HPC * 128], dt_mm, kind="ExternalInput")
    w_v = nc.dram_tensor("w_v", [C, HPC * 64], dt_mm, kind="ExternalInput")
    w_out = nc.dram_tensor("w_out", [HPC * 64, C], dt_mm, kind="ExternalInput")
    b_bcast = nc.dram_tensor("b_bcast", [128, C], F32, kind="ExternalInput")
    mask = nc.dram_tensor("mask", [128, 128], dt_mm, kind="ExternalInput")
    ones = nc.dram_tensor("ones", [128, 64], dt_mm, kind="ExternalInput")
    out = nc.dram_tensor("out", [NQ * (512 // tp), C], F32, kind="ExternalOutput")

    def mm(o, lhsT, rhs, **kw):
        nc.tensor.matmul(o, lhsT, rhs, **kw)

    n_yt = (HPC * 64 + 127) // 128   # SBUF tiles holding this core's y^T
    rw = 512 // tp

    with tile.TileContext(nc) as tc:
        with (
            tc.tile_pool(name="persist", bufs=1) as per_pool,
            tc.tile_pool(name="xt", bufs=3) as xt_pool,
            tc.tile_pool(name="pT", bufs=4) as pT_pool,
            tc.tile_pool(name="norm", bufs=3) as norm_pool,
            tc.tile_pool(name="osb", bufs=4) as o_pool,
            tc.tile_pool(name="ps_s", bufs=2, space="PSUM") as ps_s,
            tc.tile_pool(name="ps_y", bufs=2, space="PSUM") as ps_y,
            tc.tile_pool(name="ps_acc", bufs=2, space="PSUM") as ps_acc,
            tc.tile_pool(name="dram", bufs=1, space="DRAM") as dram_pool,
        ):
          for _rep in range(reps):
            # emit only wqk[0] before the first x^T chunk so the first
            # matmul's inputs are at the head of the DMA queues
            wqk_sb = []
            t = per_pool.tile([128, HPC * 128], dt_mm, name="wqk0", tag="wqk0")
            nc.sync.dma_start(t[:], w_qk[0:128, :])
            wqk_sb.append(t)
            wv_sb = []
            qkT_sb = [per_pool.tile([128, 2 * T], dt_mm, name=f"qkT{hp}", tag=f"qkT{hp}")
                      for hp in range(HPC // 2)]
            v_sb = [per_pool.tile([128, HPC * 65], dt_mm, name=f"v{mt}", tag=f"v{mt}")
                    for mt in range(TT)]
            yT_sb = [per_pool.tile([128, T], dt_mm, name=f"yT{i}", tag=f"yT{i}")
                     for i in range(n_yt)]
            rs_in = [dram_pool.tile([512, C], BF16, name=f"rsi{qc}", tag=f"rsi{qc}")
                     for qc in range(NQ)]
            rs_out = [dram_pool.tile([rw, C], BF16, name=f"rso{qc}", tag=f"rso{qc}")
                      for qc in range(NQ)]

            # schedule: proj 0, proj 1, att 1, proj 2, att 2, proj 3,
            # att 3, att 0 -- the last attention chunk is the cheapest so
            # its ReduceScatter tail is minimal.
            steps = [("proj", n) for n in range(NQ)]
            steps += [("att", n) for n in
                      (list(range(1, NQ)) + [0] if NQ > 1 else [0])]
            for kind, n in steps:
              if kind == "proj":
                # ---- x^T chunk load + qk/v projections ---------------
                xt_chunk = []
                for k in range(KT):
                    t = xt_pool.tile([128, 512], dt_mm, name=f"xt{k}", tag=f"xt{k}")
                    nc.sync.dma_start(
                        t[:], xT[128 * k:128 * (k + 1), 512 * n:512 * (n + 1)])
                    xt_chunk.append(t)
                    if n == 0 and len(wqk_sb) == k + 1 and k + 1 < KT:
                        t2 = per_pool.tile([128, HPC * 128], dt_mm,
                                           name=f"wqk{k+1}", tag=f"wqk{k+1}")
                        nc.sync.dma_start(t2[:], w_qk[128 * (k+1):128 * (k + 2), :])
                        wqk_sb.append(t2)
                for m in range(HPC):
                    hp, is_k = divmod(m, 2)
                    acc = ps_acc.tile([128, 512], F32, name="acc", tag="acc")
                    for k in range(KT):
                        mm(acc[:], wqk_sb[k][:, 128 * m:128 * (m + 1)], xt_chunk[k][:],
                           start=(k == 0), stop=(k == KT - 1))
                    off = (T if is_k else 0) + 512 * n
                    # alternate eviction engines so psum slots free faster
                    if m % 2 == 0:
                        nc.scalar.copy(qkT_sb[hp][:, off:off + 512], acc[:])
                    else:
                        nc.vector.tensor_copy(qkT_sb[hp][:, off:off + 512], acc[:])
                if n == 0:
                    for k in range(KT):
                        t = per_pool.tile([128, HPC * 64], dt_mm, name=f"wv{k}",
                                          tag=f"wv{k}")
                        nc.sync.dma_start(t[:], w_v[128 * k:128 * (k + 1), :])
                        wv_sb.append(t)
                    ones_sb = per_pool.tile([128, 64], dt_mm, name="ones", tag="ones")
                    nc.sync.dma_start(ones_sb[:], ones[:, :])
                for j in range(4):
                    mt = 4 * n + j
                    acc = ps_acc.tile([128, HPC * 64], F32, name="acc", tag="acc")
                    for k in range(KT):
                        mm(acc[:], xt_chunk[k][:, 128 * j:128 * (j + 1)], wv_sb[k][:],
                           start=(k == 0), stop=(k == KT - 1))
                    vt = v_sb[mt]
                    vsrc = acc[:].rearrange("p (h e) -> p h e", e=64)
                    vdst = vt[:].rearrange("p (h e) -> p h e", e=65)[:, :, 0:64]
                    nc.vector.tensor_copy(vdst, vsrc)
                    nc.vector.tensor_copy(
                        vt[:].rearrange("p (h e) -> p h e", e=65)[:, :, 64:65],
                        ones_sb[:, 0:HPC].rearrange("p (h e) -> p h e", e=1))
                if n == 0:
                    msk_sb = per_pool.tile([128, 128], dt_mm, name="mask", tag="mask")
                    nc.sync.dma_start(msk_sb[:], mask[:, :])
                    bb_sb = per_pool.tile([128, C], F32, name="bb", tag="bb")
                    nc.sync.dma_start(bb_sb[:], b_bcast[:, :])
                    wout_sb = []
                    for k in range(n_yt):
                        rows = min(128, HPC * 64 - 128 * k)
                        t = per_pool.tile([rows, C], dt_mm, name=f"wout{k}",
                                          tag=f"wout{k}")
                        nc.sync.dma_start(t[:], w_out[128 * k:128 * k + rows, :])
                        wout_sb.append(t)

                continue
              else:
                # ---- attention for query chunk qc = n ----------------
                qc = n
                for h in range(HPC):
                    hp, half = divmod(h, 2)
                    base = 64 * half
                    qT = qkT_sb[hp][base:base + 64, 0:T]
                    kT = qkT_sb[hp][base:base + 64, T:2 * T]
                    y_acc = ps_y.tile([128, 512], F32, name="y", tag="y")
                    n_kt = 4 * qc + 4
                    # non-diagonal tiles in pairs (one exp per pair)
                    kt = 0
                    first = True
                    while kt < 4 * qc:
                        s_ps = ps_s.tile([128, 1024], F32, name="s", tag="s")
                        pT = pT_pool.tile([128, 1024], dt_mm, name="p", tag="p")
                        for half_i in range(2):
                            mm(s_ps[:, 512 * half_i:512 * (half_i + 1)],
                               kT[:, 128 * (kt + half_i):128 * (kt + half_i + 1)],
                               qT[:, 512 * qc:512 * (qc + 1)],
                               start=True, stop=True)
                        nc.scalar.activation(pT[:], s_ps[:], Exp, scale=0.125)
                        for half_i in range(2):
                            mm(y_acc[0:65, :], v_sb[kt + half_i][:, 65 * h:65 * h + 65],
                               pT[:, 512 * half_i:512 * (half_i + 1)],
                               start=first, stop=False)
                            first = False
                        kt += 2
                    # diagonal tiles: restrict to valid columns
                    for i in range(4):
                        ktd = 4 * qc + i
                        lo = 128 * i
                        s_ps = ps_s.tile([128, 1024], F32, name="s", tag="s")
                        pT = pT_pool.tile([128, 1024], dt_mm, name="p", tag="p")
                        mm(s_ps[:, lo:512], kT[:, 128 * ktd:128 * (ktd + 1)],
                           qT[:, 512 * qc + lo:512 * (qc + 1)],
                           start=True, stop=True)
                        nc.scalar.activation(pT[:, lo:512], s_ps[:, lo:512],
                                             Exp, scale=0.125)
                        nc.vector.tensor_mul(
                            pT[:, lo:lo + 128], pT[:, lo:lo + 128], msk_sb[:])
                        mm(y_acc[0:65, lo:512], v_sb[ktd][:, 65 * h:65 * h + 65],
                           pT[:, lo:512],
                           start=first, stop=(i == 3))
                        first = False
                    # normalize: row 64 of y_acc is the denominator.
                    # reciprocal -> bf16 -> partition-broadcast via a tiny
                    # K=1 matmul (ones outer product) -- avoids the DRAM
                    # DMA round-trip.
                    r_sb = norm_pool.tile([1, 512], F32, name="r", tag="r")
                    nc.vector.reciprocal(r_sb[:], y_acc[64:65, :])
                    rb_sb = norm_pool.tile([1, 512], BF16, name="rb", tag="rb")
                    nc.vector.tensor_copy(rb_sb[:], r_sb[:])
                    # broadcast 1/den to 64 partitions inside y_acc's own
                    # (partition-padded) PSUM bank, rows 64:128
                    nc.tensor.matmul(y_acc[64:128, 0:512],
                                     ones_sb[0:1, 0:64], rb_sb[:],
                                     start=True, stop=True)
                    rbs = norm_pool.tile([64, 512], F32, name="rbs", tag="rbs")
                    nc.scalar.copy(rbs[:], y_acc[64:128, 0:512])
                    ti, po = divmod(64 * h, 128)
                    nc.vector.tensor_mul(
                        yT_sb[ti][po:po + 64, 512 * qc:512 * (qc + 1)],
                        y_acc[0:64, :], rbs[:])

                # ---- out-proj for this chunk + ReduceScatter ---------
                for j in range(4):
                    m = 4 * qc + j
                    for nn in range(C // 512):
                        acc = ps_acc.tile([128, 512], F32, name="acc", tag="acc")
                        for k in range(n_yt):
                            mm(acc[:], yT_sb[k][:, 128 * m:128 * (m + 1)],
                               wout_sb[k][:, 512 * nn:512 * (nn + 1)],
                               start=(k == 0), stop=(k == n_yt - 1))
                        po_sb = o_pool.tile([128, 512], BF16, name="po", tag="po")
                        nc.vector.tensor_add(po_sb[:], acc[:],
                                             bb_sb[:, 512 * nn:512 * (nn + 1)])
                        nc.sync.dma_start(
                            rs_in[qc][128 * j:128 * (j + 1), 512 * nn:512 * (nn + 1)],
                            po_sb[:])
                if no_rs:
                    nc.sync.dma_start(rs_out[qc][:], rs_in[qc][0:rw, :])
                else:
                    nc.gpsimd.collective_compute(
                        "ReduceScatter", mybir.AluOpType.add,
                        replica_groups=cfg["groups"],
                        ins=[rs_in[qc][:].opt()], outs=[rs_out[qc][:].opt()])
                # bf16 -> f32 via a single casting SWDGE DMA
                nc.gpsimd.dma_start(
                    out[rw * qc:rw * (qc + 1), :].rearrange("p f -> () (p f)"),
                    rs_out[qc][:].rearrange("p f -> () (p f)"))
    nc.compile()
    return nc


def shard_inputs(x, w_qkv, w_out, b_out, cfg=CFG):
    import ml_dtypes
    bf16 = ml_dtypes.bfloat16
    B, T, C, H, D, tp = (cfg["B"], cfg["T"], cfg["C"], cfg["H"], cfg["D"], cfg["tp"])
    HPC = cfg["HPC"]
    x = np.asarray(x, dtype=np.float32).astype(bf16)
    w_qkv = np.asarray(w_qkv, dtype=np.float32).astype(bf16)
    w_out = np.asarray(w_out, dtype=np.float32).astype(bf16)
    b_out = np.asarray(b_out, dtype=np.float32)

    w_q, w_k, w_v = w_qkv[:, :C], w_qkv[:, C:2 * C], w_qkv[:, 2 * C:]
    kp = np.arange(128)[:, None]
    qf = np.arange(128)[None, :]
    mask = (kp <= qf).astype(bf16)
    b_bcast = np.ascontiguousarray(np.broadcast_to(b_out / tp, (128, C)))

    in_maps = []
    for c in range(cfg["n_cores"]):
        b, r = divmod(c, tp)
        heads = range(HPC * r, HPC * (r + 1))
        heads = list(heads)
        blocks = []
        for hp in range(len(heads) // 2):
            g0, g1 = heads[2 * hp], heads[2 * hp + 1]
            blocks.append(np.concatenate(
                [w_q[:, 64 * g0:64 * (g0 + 1)], w_q[:, 64 * g1:64 * (g1 + 1)]], axis=1))
            blocks.append(np.concatenate(
                [w_k[:, 64 * g0:64 * (g0 + 1)], w_k[:, 64 * g1:64 * (g1 + 1)]], axis=1))
        wqk_c = np.concatenate(blocks, axis=1)
        wv_c = np.concatenate([w_v[:, 64 * g:64 * (g + 1)] for g in heads], axis=1)
        wout_c = np.concatenate([w_out[64 * g:64 * (g + 1), :] for g in heads], axis=0)
        in_maps.append({
            "xT": np.ascontiguousarray(x[b].T),
            "w_qk": np.ascontiguousarray(wqk_c),
            "w_v": np.ascontiguousarray(wv_c),
            "w_out": np.ascontiguousarray(wout_c),
            "b_bcast": b_bcast,
            "mask": mask,
            "ones": np.ones((128, 64), dtype=bf16),
        })
    return in_maps


def assemble(results, cfg=CFG):
    B, T, C, tp, NQ = cfg["B"], cfg["T"], cfg["C"], cfg["tp"], cfg["NQ"]
    rw = 512 // tp
    out = np.empty((B, T, C), dtype=np.float32)
    for c in range(cfg["n_cores"]):
        b, r = divmod(c, tp)
        o = results[c]["out"]
        for qc in range(NQ):
            out[b, 512 * qc + rw * r:512 * qc + rw * (r + 1), :] = \
                o[rw * qc:rw * (qc + 1)]
    return out


_NC_CACHE = {}


def _get_nc(cfg_key="default", cfg=CFG):
    if cfg_key not in _NC_CACHE:
        _NC_CACHE[cfg_key] = build_nc(cfg)
    return _NC_CACHE[cfg_key]


def kernel(x, w_qkv, w_out, b_out):
    cfg = CFG
    nc = _get_nc()
    in_maps = shard_inputs(x, w_qkv, w_out, b_out, cfg)
    res = bass_utils.run_bass_kernel_spmd(
        nc, in_maps, core_ids=list(range(cfg["n_cores"])))
    return assemble(res.results, cfg)


if __name__ == "__main__":
    print("module loads ok")



# revision 14
# speedup vs baseline: 1.0555x; 1.0555x over previous
"""Causal self-attention kernel for 8 trn2 NeuronCores.

Sharding: 2 batch groups x 4 tensor-parallel ranks (Megatron-style head
split).  Core c handles batch b=c//4 and heads [4r, 4r+4) with r=c%4.
Each core:
  1. qk^T projection:   qkT[128h:(128h+128), :] = [q_h^T; k_h^T]  (64+64 rows)
  2. v projection:      v[token, 65h:65h+64], col 65h+64 = 1.0 (den trick)
  3. causal attention in s^T = [key_part, query_free] layout:
       sT = (k^T slice) matmul q^T ; p = exp(s/8) * mask ; y'T += [v|1].T p
     row 64 of y'T is the softmax denominator; normalize via reciprocal +
     partition-broadcast + multiply.
  4. partial out = y_own @ w_out[own head rows, :]  -> [2048, 1024]
  5. ReduceScatter(add) across the 4-rank group: rank r keeps the summed
     rows [512r, 512r+512); add bias -> out [512, 1024]
Host concatenates the 8 x [512, 1024] outputs into [2, 2048, 1024].
"""

import sys

for _p in ("/opt/trn_rl_repo", "/root/.axon_site", "/root/.axon_site/_ro/trn_rl_repo",
           "/root/.axon_site/_ro/pypackages"):
    if _p not in sys.path:
        sys.path.append(_p)

import numpy as np

import concourse.mybir as mybir
import concourse.tile as tile
from concourse import bacc
from concourse import bass_utils

F32 = mybir.dt.float32
BF16 = mybir.dt.bfloat16
F32R = mybir.dt.float32r


def _cfg(B=2, T=2048, C=1024, H=16, n_cores=8, tp=4):
    D = 64
    assert C == H * D
    cfg = dict(B=B, T=T, C=C, H=H, D=D, n_cores=n_cores, tp=tp)
    cfg["groups"] = [[g * tp + r for r in range(tp)] for g in range(n_cores // tp)]
    cfg["HPC"] = H // tp           # heads per core
    cfg["KT"] = C // 128           # contraction tiles for projections
    cfg["NQ"] = T // 512           # 512-wide query chunks
    cfg["TT"] = T // 128           # 128-wide token (key) tiles
    cfg["RT"] = T // tp            # output rows per core
    assert cfg["RT"] % 128 == 0 and T % 512 == 0
    return cfg


CFG = _cfg()


def build_nc(cfg=CFG, dt_mm=BF16, reps=1, no_rs=False):
    B, T, C, H, D = cfg["B"], cfg["T"], cfg["C"], cfg["H"], cfg["D"]
    HPC, KT, NQ, TT, RT = cfg["HPC"], cfg["KT"], cfg["NQ"], cfg["TT"], cfg["RT"]
    tp = cfg["tp"]
    assert HPC % 2 == 0
    Exp = mybir.ActivationFunctionType.Exp

    nc = bacc.Bacc("TRN2", target_bir_lowering=False, debug=False,
                   enable_asserts=True, num_devices=cfg["n_cores"])

    xT = nc.dram_tensor("xT", [C, T], dt_mm, kind="ExternalInput")
    w_qk = nc.dram_tensor("w_qk", [C, HPC * 128], dt_mm, kind="ExternalInput")
    w_v = nc.dram_tensor("w_v", [C, HPC * 64], dt_mm, kind="ExternalInput")
    w_out = nc.dram_tensor("w_out", [HPC * 64, C], dt_mm, kind="ExternalInput")
    b_bcast = nc.dram_tensor("b_bcast", [128, C], F32, kind="ExternalInput")
    mask = nc.dram_tensor("mask", [128, 128], dt_mm, kind="ExternalInput")
    ones = nc.dram_tensor("ones", [128, 64], dt_mm, kind="ExternalInput")
    out = nc.dram_tensor("out", [NQ * (512 // tp), C], F32, kind="ExternalOutput")

    def mm(o, lhsT, rhs, **kw):
        nc.tensor.matmul(o, lhsT, rhs, **kw)

    n_yt = (HPC * 64 + 127) // 128   # SBUF tiles holding this core's y^T
    rw = 512 // tp

    with tile.TileContext(nc) as tc:
        with (
            tc.tile_pool(name="persist", bufs=1) as per_pool,
            tc.tile_pool(name="xt", bufs=3) as xt_pool,
            tc.tile_pool(name="pT", bufs=4) as pT_pool,
            tc.tile_pool(name="norm", bufs=3) as norm_pool,
            tc.tile_pool(name="osb", bufs=4) as o_pool,
            tc.tile_pool(name="ps_s", bufs=2, space="PSUM") as ps_s,
            tc.tile_pool(name="ps_y", bufs=2, space="PSUM") as ps_y,
            tc.tile_pool(name="ps_acc", bufs=2, space="PSUM") as ps_acc,
            tc.tile_pool(name="dram", bufs=1, space="DRAM") as dram_pool,
        ):
          for _rep in range(reps):
            # emit only wqk[0] before the first x^T chunk so the first
            # matmul's inputs are at the head of the DMA queues
            wqk_sb = []
            t = per_pool.tile([128, HPC * 128], dt_mm, name="wqk0", tag="wqk0")
            nc.sync.dma_start(t[:], w_qk[0:128, :])
            wqk_sb.append(t)
            wv_sb = []
            qkT_sb = [per_pool.tile([128, 2 * T], dt_mm, name=f"qkT{hp}", tag=f"qkT{hp}")
                      for hp in range(HPC // 2)]
            v_sb = [per_pool.tile([128, HPC * 65], dt_mm, name=f"v{mt}", tag=f"v{mt}")
                    for mt in range(TT)]
            yT_sb = [per_pool.tile([128, T], dt_mm, name=f"yT{i}", tag=f"yT{i}")
                     for i in range(n_yt)]
            rs_in = [dram_pool.tile([512, C], BF16, name=f"rsi{qc}", tag=f"rsi{qc}")
                     for qc in range(NQ)]
            rs_out = [dram_pool.tile([rw, C], BF16, name=f"rso{qc}", tag=f"rso{qc}")
                      for qc in range(NQ)]

            # schedule: proj 0, proj 1, att 1, proj 2, att 2, proj 3,
            # att 3, att 0 -- the last attention chunk is the cheapest so
            # its ReduceScatter tail is minimal.
            steps = [("proj", n) for n in range(NQ)]
            steps += [("att", n) for n in
                      (list(range(1, NQ)) + [0] if NQ > 1 else [0])]
            for kind, n in steps:
              if kind == "proj":
                # ---- x^T chunk load + qk/v projections ---------------
                xt_chunk = []
                for k in range(KT):
                    t = xt_pool.tile([128, 512], dt_mm, name=f"xt{k}", tag=f"xt{k}")
                    nc.sync.dma_start(
                        t[:], xT[128 * k:128 * (k + 1), 512 * n:512 * (n + 1)])
                    xt_chunk.append(t)
                    if n == 0 and len(wqk_sb) == k + 1 and k + 1 < KT:
                        t2 = per_pool.tile([128, HPC * 128], dt_mm,
                                           name=f"wqk{k+1}", tag=f"wqk{k+1}")
                        nc.sync.dma_start(t2[:], w_qk[128 * (k+1):128 * (k + 2), :])
                        wqk_sb.append(t2)
                for m in range(HPC):
                    hp, is_k = divmod(m, 2)
                    acc = ps_acc.tile([128, 512], F32, name="acc", tag="acc")
                    for k in range(KT):
                        mm(acc[:], wqk_sb[k][:, 128 * m:128 * (m + 1)], xt_chunk[k][:],
                           start=(k == 0), stop=(k == KT - 1))
                    off = (T if is_k else 0) + 512 * n
                    # alternate eviction engines so psum slots free faster
                    if m % 2 == 0:
                        nc.scalar.copy(qkT_sb[hp][:, off:off + 512], acc[:])
                    else:
                        nc.vector.tensor_copy(qkT_sb[hp][:, off:off + 512], acc[:])
                if n == 0:
                    for k in range(KT):
                        t = per_pool.tile([128, HPC * 64], dt_mm, name=f"wv{k}",
                                          tag=f"wv{k}")
                        nc.sync.dma_start(t[:], w_v[128 * k:128 * (k + 1), :])
                        wv_sb.append(t)
                    ones_sb = per_pool.tile([128, 64], dt_mm, name="ones", tag="ones")
                    nc.sync.dma_start(ones_sb[:], ones[:, :])
                for j in range(4):
                    mt = 4 * n + j
                    acc = ps_acc.tile([128, HPC * 64], F32, name="acc", tag="acc")
                    for k in range(KT):
                        mm(acc[:], xt_chunk[k][:, 128 * j:128 * (j + 1)], wv_sb[k][:],
                           start=(k == 0), stop=(k == KT - 1))
                    vt = v_sb[mt]
                    vsrc = acc[:].rearrange("p (h e) -> p h e", e=64)
                    vdst = vt[:].rearrange("p (h e) -> p h e", e=65)[:, :, 0:64]
                    nc.vector.tensor_copy(vdst, vsrc)
                    nc.vector.tensor_copy(
                        vt[:].rearrange("p (h e) -> p h e", e=65)[:, :, 64:65],
                        ones_sb[:, 0:HPC].rearrange("p (h e) -> p h e", e=1))
                if n == 0:
                    msk_sb = per_pool.tile([128, 128], dt_mm, name="mask", tag="mask")
                    nc.sync.dma_start(msk_sb[:], mask[:, :])
                    bb_sb = per_pool.tile([128, C], F32, name="bb", tag="bb")
                    nc.sync.dma_start(bb_sb[:], b_bcast[:, :])
                    wout_sb = []
                    for k in range(n_yt):
                        rows = min(128, HPC * 64 - 128 * k)
                        t = per_pool.tile([rows, C], dt_mm, name=f"wout{k}",
                                          tag=f"wout{k}")
                        nc.sync.dma_start(t[:], w_out[128 * k:128 * k + rows, :])
                        wout_sb.append(t)

                continue
              else:
                # ---- attention for query chunk qc = n ----------------
                qc = n
                for hp in range(HPC // 2):
                    # two heads of this qkT pair, interleaved so PE always
                    # has the other head's matmuls during an exp
                    h0, h1 = 2 * hp, 2 * hp + 1
                    qT0 = qkT_sb[hp][0:64, 0:T]
                    kT0 = qkT_sb[hp][0:64, T:2 * T]
                    qT1 = qkT_sb[hp][64:128, 0:T]
                    kT1 = qkT_sb[hp][64:128, T:2 * T]
                    y0 = ps_y.tile([128, 512], F32, name="y0", tag="y")
                    y1 = ps_y.tile([128, 512], F32, name="y1", tag="y")
                    first = True   # same accumulation pattern for y0 and y1
                    # non-diagonal tiles in pairs (one exp per head per pair)
                    kt = 0
                    while kt < 4 * qc:
                        s0 = ps_s.tile([128, 1024], F32, name="s0", tag="s")
                        s1 = ps_s.tile([128, 1024], F32, name="s1", tag="s")
                        p0 = pT_pool.tile([128, 1024], dt_mm, name="p0", tag="p")
                        p1 = pT_pool.tile([128, 1024], dt_mm, name="p1", tag="p")
                        for s_ps, qT, kT in ((s0, qT0, kT0), (s1, qT1, kT1)):
                            for half_i in range(2):
                                mm(s_ps[:, 512 * half_i:512 * (half_i + 1)],
                                   kT[:, 128 * (kt + half_i):128 * (kt + half_i + 1)],
                                   qT[:, 512 * qc:512 * (qc + 1)],
                                   start=True, stop=True)
                        nc.scalar.activation(p0[:], s0[:], Exp, scale=0.125)
                        nc.scalar.activation(p1[:], s1[:], Exp, scale=0.125)
                        for y_acc, pT, h in ((y0, p0, h0), (y1, p1, h1)):
                            for half_i in range(2):
                                mm(y_acc[0:65, :],
                                   v_sb[kt + half_i][:, 65 * h:65 * h + 65],
                                   pT[:, 512 * half_i:512 * (half_i + 1)],
                                   start=(first and half_i == 0), stop=False)
                        first = False
                        kt += 2
                    # diagonal tiles: both heads packed into one s tile
                    # (head 0 at cols [0:512-lo], head 1 at [512:1024-lo])
                    for i in range(4):
                        ktd = 4 * qc + i
                        lo = 128 * i
                        w = 512 - lo
                        s_ps = ps_s.tile([128, 1024], F32, name="sd", tag="s")
                        pT = pT_pool.tile([128, 1024], dt_mm, name="pd", tag="p")
                        mm(s_ps[:, 0:w], kT0[:, 128 * ktd:128 * (ktd + 1)],
                           qT0[:, 512 * qc + lo:512 * (qc + 1)],
                           start=True, stop=True)
                        mm(s_ps[:, 512:512 + w], kT1[:, 128 * ktd:128 * (ktd + 1)],
                           qT1[:, 512 * qc + lo:512 * (qc + 1)],
                           start=True, stop=True)
                        src = s_ps[:].rearrange("p (n f) -> p n f", n=2)[:, :, 0:w]
                        dst = pT[:].rearrange("p (n f) -> p n f", n=2)[:, :, 0:w]
                        nc.scalar.activation(dst, src, Exp, scale=0.125)
                        nc.vector.tensor_mul(
                            pT[:, 0:128], pT[:, 0:128], msk_sb[:])
                        nc.vector.tensor_mul(
                            pT[:, 512:640], pT[:, 512:640], msk_sb[:])
                        mm(y0[0:65, lo:512], v_sb[ktd][:, 65 * h0:65 * h0 + 65],
                           pT[:, 0:w], start=first, stop=(i == 3))
                        mm(y1[0:65, lo:512], v_sb[ktd][:, 65 * h1:65 * h1 + 65],
                           pT[:, 512:512 + w], start=first, stop=(i == 3))
                        first = False
                    # normalize: row 64 of y_acc is the denominator.
                    # reciprocal -> bf16 -> partition-broadcast via a tiny
                    # K=1 matmul (ones outer product) into rows 64:128 of
                    # y's own (partition-padded) PSUM bank.
                    for y_acc, h in ((y0, h0), (y1, h1)):
                        r_sb = norm_pool.tile([1, 512], F32, name="r", tag="r")
                        nc.vector.reciprocal(r_sb[:], y_acc[64:65, :])
                        rb_sb = norm_pool.tile([1, 512], BF16, name="rb", tag="rb")
                        nc.vector.tensor_copy(rb_sb[:], r_sb[:])
                        nc.tensor.matmul(y_acc[64:128, 0:512],
                                         ones_sb[0:1, 0:64], rb_sb[:],
                                         start=True, stop=True)
                        rbs = norm_pool.tile([64, 512], F32, name="rbs", tag="rbs")
                        nc.scalar.copy(rbs[:], y_acc[64:128, 0:512])
                        ti, po = divmod(64 * h, 128)
                        nc.vector.tensor_mul(
                            yT_sb[ti][po:po + 64, 512 * qc:512 * (qc + 1)],
                            y_acc[0:64, :], rbs[:])

                # ---- out-proj for this chunk + ReduceScatter ---------
                for j in range(4):
                    m = 4 * qc + j
                    for nn in range(C // 512):
                        acc = ps_acc.tile([128, 512], F32, name="acc", tag="acc")
                        for k in range(n_yt):
                            mm(acc[:], yT_sb[k][:, 128 * m:128 * (m + 1)],
                               wout_sb[k][:, 512 * nn:512 * (nn + 1)],
                               start=(k == 0), stop=(k == n_yt - 1))
                        po_sb = o_pool.tile([128, 512], BF16, name="po", tag="po")
                        nc.vector.tensor_add(po_sb[:], acc[:],
                                             bb_sb[:, 512 * nn:512 * (nn + 1)])
                        nc.sync.dma_start(
                            rs_in[qc][128 * j:128 * (j + 1), 512 * nn:512 * (nn + 1)],
                            po_sb[:])
                if no_rs:
                    nc.sync.dma_start(rs_out[qc][:], rs_in[qc][0:rw, :])
                else:
                    nc.gpsimd.collective_compute(
                        "ReduceScatter", mybir.AluOpType.add,
                        replica_groups=cfg["groups"],
                        ins=[rs_in[qc][:].opt()], outs=[rs_out[qc][:].opt()])
                # bf16 -> f32 via a single casting SWDGE DMA
                nc.gpsimd.dma_start(
                    out[rw * qc:rw * (qc + 1), :].rearrange("p f -> () (p f)"),
                    rs_out[qc][:].rearrange("p f -> () (p f)"))
    nc.compile()
    return nc


def shard_inputs(x, w_qkv, w_out, b_out, cfg=CFG):
    import ml_dtypes
    bf16 = ml_dtypes.bfloat16
    B, T, C, H, D, tp = (cfg["B"], cfg["T"], cfg["C"], cfg["H"], cfg["D"], cfg["tp"])
    HPC = cfg["HPC"]
    x = np.asarray(x, dtype=np.float32).astype(bf16)
    w_qkv = np.asarray(w_qkv, dtype=np.float32).astype(bf16)
    w_out = np.asarray(w_out, dtype=np.float32).astype(bf16)
    b_out = np.asarray(b_out, dtype=np.float32)

    w_q, w_k, w_v = w_qkv[:, :C], w_qkv[:, C:2 * C], w_qkv[:, 2 * C:]
    kp = np.arange(128)[:, None]
    qf = np.arange(128)[None, :]
    mask = (kp <= qf).astype(bf16)
    b_bcast = np.ascontiguousarray(np.broadcast_to(b_out / tp, (128, C)))

    in_maps = []
    for c in range(cfg["n_cores"]):
        b, r = divmod(c, tp)
        heads = range(HPC * r, HPC * (r + 1))
        heads = list(heads)
        blocks = []
        for hp in range(len(heads) // 2):
            g0, g1 = heads[2 * hp], heads[2 * hp + 1]
            blocks.append(np.concatenate(
                [w_q[:, 64 * g0:64 * (g0 + 1)], w_q[:, 64 * g1:64 * (g1 + 1)]], axis=1))
            blocks.append(np.concatenate(
                [w_k[:, 64 * g0:64 * (g0 + 1)], w_k[:, 64 * g1:64 * (g1 + 1)]], axis=1))
        wqk_c = np.concatenate(blocks, axis=1)
        wv_c = np.concatenate([w_v[:, 64 * g:64 * (g + 1)] for g in heads], axis=1)
        wout_c = np.concatenate([w_out[64 * g:64 * (g + 1), :] for g in heads], axis=0)
        in_maps.append({
            "xT": np.ascontiguousarray(x[b].T),
            "w_qk": np.ascontiguousarray(wqk_c),
            "w_v": np.ascontiguousarray(wv_c),
            "w_out": np.ascontiguousarray(wout_c),
            "b_bcast": b_bcast,
            "mask": mask,
            "ones": np.ones((128, 64), dtype=bf16),
        })
    return in_maps


def assemble(results, cfg=CFG):
    B, T, C, tp, NQ = cfg["B"], cfg["T"], cfg["C"], cfg["tp"], cfg["NQ"]
    rw = 512 // tp
    out = np.empty((B, T, C), dtype=np.float32)
    for c in range(cfg["n_cores"]):
        b, r = divmod(c, tp)
        o = results[c]["out"]
        for qc in range(NQ):
            out[b, 512 * qc + rw * r:512 * qc + rw * (r + 1), :] = \
                o[rw * qc:rw * (qc + 1)]
    return out


_NC_CACHE = {}


def _get_nc(cfg_key="default", cfg=CFG):
    if cfg_key not in _NC_CACHE:
        _NC_CACHE[cfg_key] = build_nc(cfg)
    return _NC_CACHE[cfg_key]


def kernel(x, w_qkv, w_out, b_out):
    cfg = CFG
    nc = _get_nc()
    in_maps = shard_inputs(x, w_qkv, w_out, b_out, cfg)
    res = bass_utils.run_bass_kernel_spmd(
        nc, in_maps, core_ids=list(range(cfg["n_cores"])))
    return assemble(res.results, cfg)


if __name__ == "__main__":
    print("module loads ok")



# revision 22
# speedup vs baseline: 1.0584x; 1.0028x over previous
"""Causal self-attention kernel for 8 trn2 NeuronCores.

Sharding: 2 batch groups x 4 tensor-parallel ranks (Megatron-style head
split).  Core c handles batch b=c//4 and heads [4r, 4r+4) with r=c%4.
Each core:
  1. qk^T projection:   qkT[128h:(128h+128), :] = [q_h^T; k_h^T]  (64+64 rows)
  2. v projection:      v[token, 65h:65h+64], col 65h+64 = 1.0 (den trick)
  3. causal attention in s^T = [key_part, query_free] layout:
       sT = (k^T slice) matmul q^T ; p = exp(s/8) * mask ; y'T += [v|1].T p
     row 64 of y'T is the softmax denominator; normalize via reciprocal +
     partition-broadcast + multiply.
  4. partial out = y_own @ w_out[own head rows, :]  -> [2048, 1024]
  5. ReduceScatter(add) across the 4-rank group: rank r keeps the summed
     rows [512r, 512r+512); add bias -> out [512, 1024]
Host concatenates the 8 x [512, 1024] outputs into [2, 2048, 1024].
"""

import sys

for _p in ("/opt/trn_rl_repo", "/root/.axon_site", "/root/.axon_site/_ro/trn_rl_repo",
           "/root/.axon_site/_ro/pypackages"):
    if _p not in sys.path:
        sys.path.append(_p)

import numpy as np

import concourse.mybir as mybir
import concourse.tile as tile
from concourse import bacc
from concourse import bass_utils

F32 = mybir.dt.float32
BF16 = mybir.dt.bfloat16
F32R = mybir.dt.float32r


def _cfg(B=2, T=2048, C=1024, H=16, n_cores=8, tp=4):
    D = 64
    assert C == H * D
    cfg = dict(B=B, T=T, C=C, H=H, D=D, n_cores=n_cores, tp=tp)
    cfg["groups"] = [[g * tp + r for r in range(tp)] for g in range(n_cores // tp)]
    cfg["HPC"] = H // tp           # heads per core
    cfg["KT"] = C // 128           # contraction tiles for projections
    cfg["NQ"] = T // 512           # 512-wide query chunks
    cfg["TT"] = T // 128           # 128-wide token (key) tiles
    cfg["RT"] = T // tp            # output rows per core
    assert cfg["RT"] % 128 == 0 and T % 512 == 0
    return cfg


CFG = _cfg()


def build_nc(cfg=CFG, dt_mm=BF16, reps=1, no_rs=False, interleave=False,
             evict_dve=False, pt_bufs=4, rs_merge=True):
    B, T, C, H, D = cfg["B"], cfg["T"], cfg["C"], cfg["H"], cfg["D"]
    HPC, KT, NQ, TT, RT = cfg["HPC"], cfg["KT"], cfg["NQ"], cfg["TT"], cfg["RT"]
    tp = cfg["tp"]
    assert HPC % 2 == 0
    Exp = mybir.ActivationFunctionType.Exp

    nc = bacc.Bacc("TRN2", target_bir_lowering=False, debug=False,
                   enable_asserts=True, num_devices=cfg["n_cores"])

    xT = nc.dram_tensor("xT", [C, T], dt_mm, kind="ExternalInput")
    w_qk = nc.dram_tensor("w_qk", [C, HPC * 128], dt_mm, kind="ExternalInput")
    w_v = nc.dram_tensor("w_v", [C, HPC * 64], dt_mm, kind="ExternalInput")
    w_out = nc.dram_tensor("w_out", [HPC * 64, C], dt_mm, kind="ExternalInput")
    b_bcast = nc.dram_tensor("b_bcast", [128, C], F32, kind="ExternalInput")
    mask = nc.dram_tensor("mask", [128, 128], dt_mm, kind="ExternalInput")
    ones = nc.dram_tensor("ones", [128, 64], dt_mm, kind="ExternalInput")
    out = nc.dram_tensor("out", [NQ * (512 // tp), C], F32, kind="ExternalOutput")

    def mm(o, lhsT, rhs, **kw):
        nc.tensor.matmul(o, lhsT, rhs, **kw)

    n_yt = (HPC * 64 + 127) // 128   # SBUF tiles holding this core's y^T
    rw = 512 // tp

    with tile.TileContext(nc) as tc:
        with (
            tc.tile_pool(name="persist", bufs=1) as per_pool,
            tc.tile_pool(name="xt", bufs=3) as xt_pool,
            tc.tile_pool(name="pT", bufs=pt_bufs) as pT_pool,
            tc.tile_pool(name="norm", bufs=3) as norm_pool,
            tc.tile_pool(name="osb", bufs=4) as o_pool,
            tc.tile_pool(name="ps_s", bufs=2, space="PSUM") as ps_s,
            tc.tile_pool(name="ps_y", bufs=2, space="PSUM") as ps_y,
            tc.tile_pool(name="ps_acc", bufs=2, space="PSUM") as ps_acc,
            tc.tile_pool(name="dram", bufs=1, space="DRAM") as dram_pool,
        ):
          for _rep in range(reps):
            # emit only wqk[0] before the first x^T chunk so the first
            # matmul's inputs are at the head of the DMA queues
            wqk_sb = []
            t = per_pool.tile([128, HPC * 128], dt_mm, name="wqk0", tag="wqk0")
            nc.sync.dma_start(t[:], w_qk[0:128, :])
            wqk_sb.append(t)
            wv_sb = []
            qkT_sb = [per_pool.tile([128, 2 * T], dt_mm, name=f"qkT{hp}", tag=f"qkT{hp}")
                      for hp in range(HPC // 2)]
            v_sb = [per_pool.tile([128, HPC * 65], dt_mm, name=f"v{mt}", tag=f"v{mt}")
                    for mt in range(TT)]
            yT_sb = [per_pool.tile([128, T], dt_mm, name=f"yT{i}", tag=f"yT{i}")
                     for i in range(n_yt)]
            rs_in = [dram_pool.tile([512, C], BF16, name=f"rsi{qc}", tag=f"rsi{qc}")
                     for qc in range(NQ)]
            rs_out = [dram_pool.tile([rw, C], BF16, name=f"rso{qc}", tag=f"rso{qc}")
                      for qc in range(NQ)]
            rs_in2 = [dram_pool.tile([1024, C], BF16, name=f"rsi2{i}", tag=f"rsi2{i}")
                      for i in range(2)]
            rs_out2 = [dram_pool.tile([2 * rw, C], BF16, name=f"rso2{i}", tag=f"rso2{i}")
                       for i in range(2)]

            # schedule: proj 0, proj 1, att 1, proj 2, att 2, proj 3,
            # att 3, att 0 -- the last attention chunk is the cheapest so
            # its ReduceScatter tail is minimal.
            if interleave and NQ == 4:
                steps = [("proj", 0), ("proj", 1), ("att", 1), ("proj", 2),
                         ("att", 2), ("proj", 3), ("att", 3), ("att", 0)]
            else:
                steps = [("proj", n) for n in range(NQ)]
                steps += [("att", n) for n in
                          (list(range(1, NQ)) + [0] if NQ > 1 else [0])]
            for kind, n in steps:
              if kind == "proj":
                # ---- x^T chunk load + qk/v projections ---------------
                xt_chunk = []
                for k in range(KT):
                    t = xt_pool.tile([128, 512], dt_mm, name=f"xt{k}", tag=f"xt{k}")
                    nc.sync.dma_start(
                        t[:], xT[128 * k:128 * (k + 1), 512 * n:512 * (n + 1)])
                    xt_chunk.append(t)
                    if n == 0 and len(wqk_sb) == k + 1 and k + 1 < KT:
                        t2 = per_pool.tile([128, HPC * 128], dt_mm,
                                           name=f"wqk{k+1}", tag=f"wqk{k+1}")
                        nc.sync.dma_start(t2[:], w_qk[128 * (k+1):128 * (k + 2), :])
                        wqk_sb.append(t2)
                for m in range(HPC):
                    hp, is_k = divmod(m, 2)
                    acc = ps_acc.tile([128, 512], F32, name="acc", tag="acc")
                    for k in range(KT):
                        mm(acc[:], wqk_sb[k][:, 128 * m:128 * (m + 1)], xt_chunk[k][:],
                           start=(k == 0), stop=(k == KT - 1))
                    off = (T if is_k else 0) + 512 * n
                    # alternate eviction engines so psum slots free faster
                    if m % 2 == 0 and not evict_dve:
                        nc.scalar.copy(qkT_sb[hp][:, off:off + 512], acc[:])
                    else:
                        nc.vector.tensor_copy(qkT_sb[hp][:, off:off + 512], acc[:])
                if n == 0:
                    for k in range(KT):
                        t = per_pool.tile([128, HPC * 64], dt_mm, name=f"wv{k}",
                                          tag=f"wv{k}")
                        nc.sync.dma_start(t[:], w_v[128 * k:128 * (k + 1), :])
                        wv_sb.append(t)
                    ones_sb = per_pool.tile([128, 64], dt_mm, name="ones", tag="ones")
                    nc.sync.dma_start(ones_sb[:], ones[:, :])
                for j in range(4):
                    mt = 4 * n + j
                    acc = ps_acc.tile([128, HPC * 64], F32, name="acc", tag="acc")
                    for k in range(KT):
                        mm(acc[:], xt_chunk[k][:, 128 * j:128 * (j + 1)], wv_sb[k][:],
                           start=(k == 0), stop=(k == KT - 1))
                    vt = v_sb[mt]
                    vsrc = acc[:].rearrange("p (h e) -> p h e", e=64)
                    vdst = vt[:].rearrange("p (h e) -> p h e", e=65)[:, :, 0:64]
                    nc.vector.tensor_copy(vdst, vsrc)
                    nc.vector.tensor_copy(
                        vt[:].rearrange("p (h e) -> p h e", e=65)[:, :, 64:65],
                        ones_sb[:, 0:HPC].rearrange("p (h e) -> p h e", e=1))
                if n == 0:
                    msk_sb = per_pool.tile([128, 128], dt_mm, name="mask", tag="mask")
                    nc.sync.dma_start(msk_sb[:], mask[:, :])
                    bb_sb = per_pool.tile([128, C], F32, name="bb", tag="bb")
                    nc.sync.dma_start(bb_sb[:], b_bcast[:, :])
                    wout_sb = []
                    for k in range(n_yt):
                        rows = min(128, HPC * 64 - 128 * k)
                        t = per_pool.tile([rows, C], dt_mm, name=f"wout{k}",
                                          tag=f"wout{k}")
                        nc.sync.dma_start(t[:], w_out[128 * k:128 * k + rows, :])
                        wout_sb.append(t)

                continue
              else:
                # ---- attention for query chunk qc = n ----------------
                qc = n
                for hp in range(HPC // 2):
                    # two heads of this qkT pair, interleaved so PE always
                    # has the other head's matmuls during an exp
                    h0, h1 = 2 * hp, 2 * hp + 1
                    qT0 = qkT_sb[hp][0:64, 0:T]
                    kT0 = qkT_sb[hp][0:64, T:2 * T]
                    qT1 = qkT_sb[hp][64:128, 0:T]
                    kT1 = qkT_sb[hp][64:128, T:2 * T]
                    y0 = ps_y.tile([128, 512], F32, name="y0", tag="y")
                    y1 = ps_y.tile([128, 512], F32, name="y1", tag="y")
                    first = True   # same accumulation pattern for y0 and y1
                    # non-diagonal tiles in pairs (one exp per head per pair)
                    kt = 0
                    while kt < 4 * qc:
                        s0 = ps_s.tile([128, 1024], F32, name="s0", tag="s")
                        s1 = ps_s.tile([128, 1024], F32, name="s1", tag="s")
                        p0 = pT_pool.tile([128, 1024], dt_mm, name="p0", tag="p")
                        p1 = pT_pool.tile([128, 1024], dt_mm, name="p1", tag="p")
                        for s_ps, qT, kT in ((s0, qT0, kT0), (s1, qT1, kT1)):
                            for half_i in range(2):
                                mm(s_ps[:, 512 * half_i:512 * (half_i + 1)],
                                   kT[:, 128 * (kt + half_i):128 * (kt + half_i + 1)],
                                   qT[:, 512 * qc:512 * (qc + 1)],
                                   start=True, stop=True)
                        nc.scalar.activation(p0[:], s0[:], Exp, scale=0.125)
                        nc.scalar.activation(p1[:], s1[:], Exp, scale=0.125)
                        for y_acc, pT, h in ((y0, p0, h0), (y1, p1, h1)):
                            for half_i in range(2):
                                mm(y_acc[0:65, :],
                                   v_sb[kt + half_i][:, 65 * h:65 * h + 65],
                                   pT[:, 512 * half_i:512 * (half_i + 1)],
                                   start=(first and half_i == 0), stop=False)
                        first = False
                        kt += 2
                    # diagonal tiles: both heads packed into one s tile
                    # (head 0 at cols [0:512-lo], head 1 at [512:1024-lo])
                    for i in range(4):
                        ktd = 4 * qc + i
                        lo = 128 * i
                        w = 512 - lo
                        s_ps = ps_s.tile([128, 1024], F32, name="sd", tag="s")
                        pT = pT_pool.tile([128, 1024], dt_mm, name="pd", tag="p")
                        mm(s_ps[:, 0:w], kT0[:, 128 * ktd:128 * (ktd + 1)],
                           qT0[:, 512 * qc + lo:512 * (qc + 1)],
                           start=True, stop=True)
                        mm(s_ps[:, 512:512 + w], kT1[:, 128 * ktd:128 * (ktd + 1)],
                           qT1[:, 512 * qc + lo:512 * (qc + 1)],
                           start=True, stop=True)
                        src = s_ps[:].rearrange("p (n f) -> p n f", n=2)[:, :, 0:w]
                        dst = pT[:].rearrange("p (n f) -> p n f", n=2)[:, :, 0:w]
                        nc.scalar.activation(dst, src, Exp, scale=0.125)
                        nc.vector.tensor_mul(
                            pT[:, 0:128], pT[:, 0:128], msk_sb[:])
                        nc.vector.tensor_mul(
                            pT[:, 512:640], pT[:, 512:640], msk_sb[:])
                        mm(y0[0:65, lo:512], v_sb[ktd][:, 65 * h0:65 * h0 + 65],
                           pT[:, 0:w], start=first, stop=(i == 3))
                        mm(y1[0:65, lo:512], v_sb[ktd][:, 65 * h1:65 * h1 + 65],
                           pT[:, 512:512 + w], start=first, stop=(i == 3))
                        first = False
                    # normalize: row 64 of y_acc is the denominator.
                    # reciprocal -> bf16 -> partition-broadcast via a tiny
                    # K=1 matmul (ones outer product) into rows 64:128 of
                    # y's own (partition-padded) PSUM bank.
                    for y_acc, h in ((y0, h0), (y1, h1)):
                        r_sb = norm_pool.tile([1, 512], F32, name="r", tag="r")
                        nc.vector.reciprocal(r_sb[:], y_acc[64:65, :])
                        rb_sb = norm_pool.tile([1, 512], BF16, name="rb", tag="rb")
                        nc.vector.tensor_copy(rb_sb[:], r_sb[:])
                        nc.tensor.matmul(y_acc[64:128, 0:512],
                                         ones_sb[0:1, 0:64], rb_sb[:],
                                         start=True, stop=True)
                        rbs = norm_pool.tile([64, 512], F32, name="rbs", tag="rbs")
                        nc.scalar.copy(rbs[:], y_acc[64:128, 0:512])
                        ti, po = divmod(64 * h, 128)
                        nc.vector.tensor_mul(
                            yT_sb[ti][po:po + 64, 512 * qc:512 * (qc + 1)],
                            y_acc[0:64, :], rbs[:])

                # ---- out-proj for this chunk + ReduceScatter ---------
                if rs_merge:
                    # chunks are RS'd in pairs: att order (1,2,3,0) ->
                    # pair 0 = chunks {1,2}, pair 1 = chunks {3,0}.
                    # rows interleaved per 128-row j-tile so rank r's
                    # scatter block [256r:256r+256] = [chunk_a j=r;
                    # chunk_b j=r].
                    pair, slot = {1: (0, 0), 2: (0, 1),
                                  3: (1, 0), 0: (1, 1)}[qc]
                for j in range(4):
                    m = 4 * qc + j
                    for nn in range(C // 512):
                        acc = ps_acc.tile([128, 512], F32, name="acc", tag="acc")
                        for k in range(n_yt):
                            mm(acc[:], yT_sb[k][:, 128 * m:128 * (m + 1)],
                               wout_sb[k][:, 512 * nn:512 * (nn + 1)],
                               start=(k == 0), stop=(k == n_yt - 1))
                        po_sb = o_pool.tile([128, 512], BF16, name="po", tag="po")
                        nc.vector.tensor_add(po_sb[:], acc[:],
                                             bb_sb[:, 512 * nn:512 * (nn + 1)])
                        if rs_merge:
                            dst = rs_in2[pair][256 * j + 128 * slot:
                                               256 * j + 128 * slot + 128,
                                               512 * nn:512 * (nn + 1)]
                        else:
                            dst = rs_in[qc][128 * j:128 * (j + 1),
                                            512 * nn:512 * (nn + 1)]
                        nc.sync.dma_start(dst, po_sb[:])
                if rs_merge:
                    if slot == 1:   # second chunk of the pair: run the RS
                        nc.gpsimd.collective_compute(
                            "ReduceScatter", mybir.AluOpType.add,
                            replica_groups=cfg["groups"],
                            ins=[rs_in2[pair][:].opt()],
                            outs=[rs_out2[pair][:].opt()])
                        qa, qb = ((1, 2), (3, 0))[pair]
                        for half, qx in ((0, qa), (1, qb)):
                            nc.gpsimd.dma_start(
                                out[rw * qx:rw * (qx + 1), :]
                                .rearrange("p f -> () (p f)"),
                                rs_out2[pair][128 * half:128 * half + rw, :]
                                .rearrange("p f -> () (p f)"))
                    continue
                if no_rs:
                    nc.sync.dma_start(rs_out[qc][:], rs_in[qc][0:rw, :])
                else:
                    nc.gpsimd.collective_compute(
                        "ReduceScatter", mybir.AluOpType.add,
                        replica_groups=cfg["groups"],
                        ins=[rs_in[qc][:].opt()], outs=[rs_out[qc][:].opt()])
                # bf16 -> f32 via a single casting SWDGE DMA
                nc.gpsimd.dma_start(
                    out[rw * qc:rw * (qc + 1), :].rearrange("p f -> () (p f)"),
                    rs_out[qc][:].rearrange("p f -> () (p f)"))
    nc.compile()
    return nc


def shard_inputs(x, w_qkv, w_out, b_out, cfg=CFG):
    import ml_dtypes
    bf16 = ml_dtypes.bfloat16
    B, T, C, H, D, tp = (cfg["B"], cfg["T"], cfg["C"], cfg["H"], cfg["D"], cfg["tp"])
    HPC = cfg["HPC"]
    x = np.asarray(x, dtype=np.float32).astype(bf16)
    w_qkv = np.asarray(w_qkv, dtype=np.float32).astype(bf16)
    w_out = np.asarray(w_out, dtype=np.float32).astype(bf16)
    b_out = np.asarray(b_out, dtype=np.float32)

    w_q, w_k, w_v = w_qkv[:, :C], w_qkv[:, C:2 * C], w_qkv[:, 2 * C:]
    kp = np.arange(128)[:, None]
    qf = np.arange(128)[None, :]
    mask = (kp <= qf).astype(bf16)
    b_bcast = np.ascontiguousarray(np.broadcast_to(b_out / tp, (128, C)))

    in_maps = []
    for c in range(cfg["n_cores"]):
        b, r = divmod(c, tp)
        heads = range(HPC * r, HPC * (r + 1))
        heads = list(heads)
        blocks = []
        for hp in range(len(heads) // 2):
            g0, g1 = heads[2 * hp], heads[2 * hp + 1]
            blocks.append(np.concatenate(
                [w_q[:, 64 * g0:64 * (g0 + 1)], w_q[:, 64 * g1:64 * (g1 + 1)]], axis=1))
            blocks.append(np.concatenate(
                [w_k[:, 64 * g0:64 * (g0 + 1)], w_k[:, 64 * g1:64 * (g1 + 1)]], axis=1))
        wqk_c = np.concatenate(blocks, axis=1)
        wv_c = np.concatenate([w_v[:, 64 * g:64 * (g + 1)] for g in heads], axis=1)
        wout_c = np.concatenate([w_out[64 * g:64 * (g + 1), :] for g in heads], axis=0)
        in_maps.append({
            "xT": np.ascontiguousarray(x[b].T),
            "w_qk": np.ascontiguousarray(wqk_c),
            "w_v": np.ascontiguousarray(wv_c),
            "w_out": np.ascontiguousarray(wout_c),
            "b_bcast": b_bcast,
            "mask": mask,
            "ones": np.ones((128, 64), dtype=bf16),
        })
    return in_maps


def assemble(results, cfg=CFG):
    B, T, C, tp, NQ = cfg["B"], cfg["T"], cfg["C"], cfg["tp"], cfg["NQ"]
    rw = 512 // tp
    out = np.empty((B, T, C), dtype=np.float32)
    for c in range(cfg["n_cores"]):
        b, r = divmod(c, tp)
        o = results[c]["out"]
        for qc in range(NQ):
            out[b, 512 * qc + rw * r:512 * qc + rw * (r + 1), :] = \
                o[rw * qc:rw * (qc + 1)]
    return out


_NC_CACHE = {}


def _get_nc(cfg_key="default", cfg=CFG):
    if cfg_key not in _NC_CACHE:
        _NC_CACHE[cfg_key] = build_nc(cfg)
    return _NC_CACHE[cfg_key]


def kernel(x, w_qkv, w_out, b_out):
    cfg = CFG
    nc = _get_nc()
    in_maps = shard_inputs(x, w_qkv, w_out, b_out, cfg)
    res = bass_utils.run_bass_kernel_spmd(
        nc, in_maps, core_ids=list(range(cfg["n_cores"])))
    return assemble(res.results, cfg)


if __name__ == "__main__":
    print("module loads ok")



# revision 25
# speedup vs baseline: 11466.2340x; 10833.3635x over previous
"""Causal self-attention kernel for 8 trn2 NeuronCores.

Sharding: 2 batch groups x 4 tensor-parallel ranks (Megatron-style head
split).  Core c handles batch b=c//4 and heads [4r, 4r+4) with r=c%4.
Each core:
  1. qk^T projection:   qkT[128h:(128h+128), :] = [q_h^T; k_h^T]  (64+64 rows)
  2. v projection:      v[token, 65h:65h+64], col 65h+64 = 1.0 (den trick)
  3. causal attention in s^T = [key_part, query_free] layout:
       sT = (k^T slice) matmul q^T ; p = exp(s/8) * mask ; y'T += [v|1].T p
     row 64 of y'T is the softmax denominator; normalize via reciprocal +
     partition-broadcast + multiply.
  4. partial out = y_own @ w_out[own head rows, :]  -> [2048, 1024]
  5. ReduceScatter(add) across the 4-rank group: rank r keeps the summed
     rows [512r, 512r+512); add bias -> out [512, 1024]
Host concatenates the 8 x [512, 1024] outputs into [2, 2048, 1024].
"""

import sys

for _p in ("/opt/trn_rl_repo", "/root/.axon_site", "/root/.axon_site/_ro/trn_rl_repo",
           "/root/.axon_site/_ro/pypackages"):
    if _p not in sys.path:
        sys.path.append(_p)

import numpy as np

import concourse.mybir as mybir
import concourse.tile as tile
from concourse import bacc
from concourse import bass_utils

F32 = mybir.dt.float32
BF16 = mybir.dt.bfloat16
F32R = mybir.dt.float32r


def _cfg(B=2, T=2048, C=1024, H=16, n_cores=8, tp=4):
    D = 64
    assert C == H * D
    cfg = dict(B=B, T=T, C=C, H=H, D=D, n_cores=n_cores, tp=tp)
    cfg["groups"] = [[g * tp + r for r in range(tp)] for g in range(n_cores // tp)]
    cfg["HPC"] = H // tp           # heads per core
    cfg["KT"] = C // 128           # contraction tiles for projections
    cfg["NQ"] = T // 512           # 512-wide query chunks
    cfg["TT"] = T // 128           # 128-wide token (key) tiles
    cfg["RT"] = T // tp            # output rows per core
    assert cfg["RT"] % 128 == 0 and T % 512 == 0
    return cfg


CFG = _cfg()


def build_nc(cfg=CFG, dt_mm=BF16, reps=1, no_rs=False, interleave=False,
             evict_dve=False, pt_bufs=4, rs_merge=True, s_single=False,
             rs1=False):
    B, T, C, H, D = cfg["B"], cfg["T"], cfg["C"], cfg["H"], cfg["D"]
    HPC, KT, NQ, TT, RT = cfg["HPC"], cfg["KT"], cfg["NQ"], cfg["TT"], cfg["RT"]
    tp = cfg["tp"]
    assert HPC % 2 == 0
    Exp = mybir.ActivationFunctionType.Exp

    nc = bacc.Bacc("TRN2", target_bir_lowering=False, debug=False,
                   enable_asserts=True, num_devices=cfg["n_cores"])

    xT = nc.dram_tensor("xT", [C, T], dt_mm, kind="ExternalInput")
    w_qk = nc.dram_tensor("w_qk", [C, HPC * 128], dt_mm, kind="ExternalInput")
    w_v = nc.dram_tensor("w_v", [C, HPC * 64], dt_mm, kind="ExternalInput")
    w_out = nc.dram_tensor("w_out", [HPC * 64, C], dt_mm, kind="ExternalInput")
    b_bcast = nc.dram_tensor("b_bcast", [128, C], F32, kind="ExternalInput")
    mask = nc.dram_tensor("mask", [128, 128], dt_mm, kind="ExternalInput")
    ones = nc.dram_tensor("ones", [128, 64], dt_mm, kind="ExternalInput")
    out = nc.dram_tensor("out", [NQ * (512 // tp), C], F32, kind="ExternalOutput")

    def mm(o, lhsT, rhs, **kw):
        nc.tensor.matmul(o, lhsT, rhs, **kw)

    n_yt = (HPC * 64 + 127) // 128   # SBUF tiles holding this core's y^T
    rw = 512 // tp

    with tile.TileContext(nc) as tc:
        with (
            tc.tile_pool(name="persist", bufs=1) as per_pool,
            tc.tile_pool(name="xt", bufs=3) as xt_pool,
            tc.tile_pool(name="pT", bufs=pt_bufs) as pT_pool,
            tc.tile_pool(name="norm", bufs=3) as norm_pool,
            tc.tile_pool(name="osb", bufs=4) as o_pool,
            tc.tile_pool(name="ps_s", bufs=(4 if s_single else 2),
                         space="PSUM") as ps_s,
            tc.tile_pool(name="ps_y", bufs=2, space="PSUM") as ps_y,
            tc.tile_pool(name="ps_acc", bufs=2, space="PSUM") as ps_acc,
            tc.tile_pool(name="dram", bufs=1, space="DRAM") as dram_pool,
        ):
          for _rep in range(reps):
            # emit only wqk[0] before the first x^T chunk so the first
            # matmul's inputs are at the head of the DMA queues
            wqk_sb = []
            t = per_pool.tile([128, HPC * 128], dt_mm, name="wqk0", tag="wqk0")
            nc.sync.dma_start(t[:], w_qk[0:128, :])
            wqk_sb.append(t)
            wv_sb = []
            qkT_sb = [per_pool.tile([128, 2 * T], dt_mm, name=f"qkT{hp}", tag=f"qkT{hp}")
                      for hp in range(HPC // 2)]
            v_sb = [per_pool.tile([128, HPC * 65], dt_mm, name=f"v{mt}", tag=f"v{mt}")
                    for mt in range(TT)]
            yT_sb = [per_pool.tile([128, T], dt_mm, name=f"yT{i}", tag=f"yT{i}")
                     for i in range(n_yt)]
            rs_in = [dram_pool.tile([512, C], BF16, name=f"rsi{qc}", tag=f"rsi{qc}")
                     for qc in range(NQ)]
            rs_out = [dram_pool.tile([rw, C], BF16, name=f"rso{qc}", tag=f"rso{qc}")
                      for qc in range(NQ)]
            rs_in2 = [dram_pool.tile([1024, C], BF16, name=f"rsi2{i}", tag=f"rsi2{i}")
                      for i in range(2)]
            rs_in1 = dram_pool.tile([2048, C], BF16, name="rsi1", tag="rsi1")
            rs_out1 = dram_pool.tile([4 * rw, C], BF16, name="rso1", tag="rso1")
            rs_out2 = [dram_pool.tile([2 * rw, C], BF16, name=f"rso2{i}", tag=f"rso2{i}")
                       for i in range(2)]

            # schedule: proj 0, proj 1, att 1, proj 2, att 2, proj 3,
            # att 3, att 0 -- the last attention chunk is the cheapest so
            # its ReduceScatter tail is minimal.
            if interleave and NQ == 4:
                steps = [("proj", 0), ("proj", 1), ("att", 1), ("proj", 2),
                         ("att", 2), ("proj", 3), ("att", 3), ("att", 0)]
            else:
                steps = [("proj", n) for n in range(NQ)]
                steps += [("att", n) for n in
                          (list(range(1, NQ)) + [0] if NQ > 1 else [0])]
            for kind, n in steps:
              if kind == "proj":
                # ---- x^T chunk load + qk/v projections ---------------
                xt_chunk = []
                for k in range(KT):
                    t = xt_pool.tile([128, 512], dt_mm, name=f"xt{k}", tag=f"xt{k}")
                    nc.sync.dma_start(
                        t[:], xT[128 * k:128 * (k + 1), 512 * n:512 * (n + 1)])
                    xt_chunk.append(t)
                    if n == 0 and len(wqk_sb) == k + 1 and k + 1 < KT:
                        t2 = per_pool.tile([128, HPC * 128], dt_mm,
                                           name=f"wqk{k+1}", tag=f"wqk{k+1}")
                        nc.sync.dma_start(t2[:], w_qk[128 * (k+1):128 * (k + 2), :])
                        wqk_sb.append(t2)
                for m in range(HPC):
                    hp, is_k = divmod(m, 2)
                    acc = ps_acc.tile([128, 512], F32, name="acc", tag="acc")
                    for k in range(KT):
                        mm(acc[:], wqk_sb[k][:, 128 * m:128 * (m + 1)], xt_chunk[k][:],
                           start=(k == 0), stop=(k == KT - 1))
                    off = (T if is_k else 0) + 512 * n
                    # alternate eviction engines so psum slots free faster
                    if m % 2 == 0 and not evict_dve:
                        nc.scalar.copy(qkT_sb[hp][:, off:off + 512], acc[:])
                    else:
                        nc.vector.tensor_copy(qkT_sb[hp][:, off:off + 512], acc[:])
                if n == 0:
                    for k in range(KT):
                        t = per_pool.tile([128, HPC * 64], dt_mm, name=f"wv{k}",
                                          tag=f"wv{k}")
                        nc.sync.dma_start(t[:], w_v[128 * k:128 * (k + 1), :])
                        wv_sb.append(t)
                    ones_sb = per_pool.tile([128, 64], dt_mm, name="ones", tag="ones")
                    nc.sync.dma_start(ones_sb[:], ones[:, :])
                for j in range(4):
                    mt = 4 * n + j
                    acc = ps_acc.tile([128, HPC * 64], F32, name="acc", tag="acc")
                    for k in range(KT):
                        mm(acc[:], xt_chunk[k][:, 128 * j:128 * (j + 1)], wv_sb[k][:],
                           start=(k == 0), stop=(k == KT - 1))
                    vt = v_sb[mt]
                    vsrc = acc[:].rearrange("p (h e) -> p h e", e=64)
                    vdst = vt[:].rearrange("p (h e) -> p h e", e=65)[:, :, 0:64]
                    nc.vector.tensor_copy(vdst, vsrc)
                    nc.vector.tensor_copy(
                        vt[:].rearrange("p (h e) -> p h e", e=65)[:, :, 64:65],
                        ones_sb[:, 0:HPC].rearrange("p (h e) -> p h e", e=1))
                if n == 0:
                    msk_sb = per_pool.tile([128, 128], dt_mm, name="mask", tag="mask")
                    nc.sync.dma_start(msk_sb[:], mask[:, :])
                    bb_sb = per_pool.tile([128, C], F32, name="bb", tag="bb")
                    nc.sync.dma_start(bb_sb[:], b_bcast[:, :])
                    wout_sb = []
                    for k in range(n_yt):
                        rows = min(128, HPC * 64 - 128 * k)
                        t = per_pool.tile([rows, C], dt_mm, name=f"wout{k}",
                                          tag=f"wout{k}")
                        nc.sync.dma_start(t[:], w_out[128 * k:128 * k + rows, :])
                        wout_sb.append(t)

                continue
              else:
                # ---- attention for query chunk qc = n ----------------
                qc = n
                for hp in range(HPC // 2):
                    # two heads of this qkT pair, interleaved so PE always
                    # has the other head's matmuls during an exp
                    h0, h1 = 2 * hp, 2 * hp + 1
                    qT0 = qkT_sb[hp][0:64, 0:T]
                    kT0 = qkT_sb[hp][0:64, T:2 * T]
                    qT1 = qkT_sb[hp][64:128, 0:T]
                    kT1 = qkT_sb[hp][64:128, T:2 * T]
                    y0 = ps_y.tile([128, 512], F32, name="y0", tag="y")
                    y1 = ps_y.tile([128, 512], F32, name="y1", tag="y")
                    first = True   # same accumulation pattern for y0 and y1
                    # non-diagonal tiles in pairs (one exp per head per pair)
                    kt = 0
                    while kt < 4 * qc:
                      if s_single:
                        for sub, (qT, kT, y_acc, h) in enumerate(
                                ((qT0, kT0, y0, h0), (qT1, kT1, y1, h1))):
                            for half_i in range(2):
                                s_ps = ps_s.tile([128, 512], F32,
                                                 name="s", tag="s")
                                pT = pT_pool.tile([128, 512], dt_mm,
                                                  name="p", tag="p")
                                mm(s_ps[:],
                                   kT[:, 128 * (kt + half_i):128 * (kt + half_i + 1)],
                                   qT[:, 512 * qc:512 * (qc + 1)],
                                   start=True, stop=True)
                                nc.scalar.activation(pT[:], s_ps[:], Exp,
                                                     scale=0.125)
                                mm(y_acc[0:65, :],
                                   v_sb[kt + half_i][:, 65 * h:65 * h + 65],
                                   pT[:],
                                   start=(first and half_i == 0), stop=False)
                        first = False
                        kt += 2
                      else:
                        s0 = ps_s.tile([128, 1024], F32, name="s0", tag="s")
                        s1 = ps_s.tile([128, 1024], F32, name="s1", tag="s")
                        p0 = pT_pool.tile([128, 1024], dt_mm, name="p0", tag="p")
                        p1 = pT_pool.tile([128, 1024], dt_mm, name="p1", tag="p")
                        for s_ps, qT, kT in ((s0, qT0, kT0), (s1, qT1, kT1)):
                            for half_i in range(2):
                                mm(s_ps[:, 512 * half_i:512 * (half_i + 1)],
                                   kT[:, 128 * (kt + half_i):128 * (kt + half_i + 1)],
                                   qT[:, 512 * qc:512 * (qc + 1)],
                                   start=True, stop=True)
                        nc.scalar.activation(p0[:], s0[:], Exp, scale=0.125)
                        nc.scalar.activation(p1[:], s1[:], Exp, scale=0.125)
                        for y_acc, pT, h in ((y0, p0, h0), (y1, p1, h1)):
                            for half_i in range(2):
                                mm(y_acc[0:65, :],
                                   v_sb[kt + half_i][:, 65 * h:65 * h + 65],
                                   pT[:, 512 * half_i:512 * (half_i + 1)],
                                   start=(first and half_i == 0), stop=False)
                        first = False
                        kt += 2
                    # diagonal tiles: both heads packed into one s tile
                    # (head 0 at cols [0:512-lo], head 1 at [512:1024-lo])
                    for i in range(4):
                        ktd = 4 * qc + i
                        lo = 128 * i
                        w = 512 - lo
                        if s_single:
                            for qT, kT, y_acc, h in ((qT0, kT0, y0, h0),
                                                     (qT1, kT1, y1, h1)):
                                s_ps = ps_s.tile([128, 512], F32,
                                                 name="sd", tag="s")
                                pT = pT_pool.tile([128, 512], dt_mm,
                                                  name="pd", tag="p")
                                mm(s_ps[:, 0:w],
                                   kT[:, 128 * ktd:128 * (ktd + 1)],
                                   qT[:, 512 * qc + lo:512 * (qc + 1)],
                                   start=True, stop=True)
                                nc.scalar.activation(pT[:, 0:w], s_ps[:, 0:w],
                                                     Exp, scale=0.125)
                                nc.vector.tensor_mul(
                                    pT[:, 0:128], pT[:, 0:128], msk_sb[:])
                                mm(y_acc[0:65, lo:512],
                                   v_sb[ktd][:, 65 * h:65 * h + 65],
                                   pT[:, 0:w], start=first, stop=(i == 3))
                            first = False
                            continue
                        s_ps = ps_s.tile([128, 1024], F32, name="sd", tag="s")
                        pT = pT_pool.tile([128, 1024], dt_mm, name="pd", tag="p")
                        mm(s_ps[:, 0:w], kT0[:, 128 * ktd:128 * (ktd + 1)],
                           qT0[:, 512 * qc + lo:512 * (qc + 1)],
                           start=True, stop=True)
                        mm(s_ps[:, 512:512 + w], kT1[:, 128 * ktd:128 * (ktd + 1)],
                           qT1[:, 512 * qc + lo:512 * (qc + 1)],
                           start=True, stop=True)
                        src = s_ps[:].rearrange("p (n f) -> p n f", n=2)[:, :, 0:w]
                        dst = pT[:].rearrange("p (n f) -> p n f", n=2)[:, :, 0:w]
                        nc.scalar.activation(dst, src, Exp, scale=0.125)
                        nc.vector.tensor_mul(
                            pT[:, 0:128], pT[:, 0:128], msk_sb[:])
                        nc.vector.tensor_mul(
                            pT[:, 512:640], pT[:, 512:640], msk_sb[:])
                        mm(y0[0:65, lo:512], v_sb[ktd][:, 65 * h0:65 * h0 + 65],
                           pT[:, 0:w], start=first, stop=(i == 3))
                        mm(y1[0:65, lo:512], v_sb[ktd][:, 65 * h1:65 * h1 + 65],
                           pT[:, 512:512 + w], start=first, stop=(i == 3))
                        first = False
                    # normalize: row 64 of y_acc is the denominator.
                    # reciprocal -> bf16 -> partition-broadcast via a tiny
                    # K=1 matmul (ones outer product) into rows 64:128 of
                    # y's own (partition-padded) PSUM bank.
                    for y_acc, h in ((y0, h0), (y1, h1)):
                        r_sb = norm_pool.tile([1, 512], F32, name="r", tag="r")
                        nc.vector.reciprocal(r_sb[:], y_acc[64:65, :])
                        rb_sb = norm_pool.tile([1, 512], BF16, name="rb", tag="rb")
                        nc.vector.tensor_copy(rb_sb[:], r_sb[:])
                        nc.tensor.matmul(y_acc[64:128, 0:512],
                                         ones_sb[0:1, 0:64], rb_sb[:],
                                         start=True, stop=True)
                        rbs = norm_pool.tile([64, 512], F32, name="rbs", tag="rbs")
                        nc.scalar.copy(rbs[:], y_acc[64:128, 0:512])
                        ti, po = divmod(64 * h, 128)
                        nc.vector.tensor_mul(
                            yT_sb[ti][po:po + 64, 512 * qc:512 * (qc + 1)],
                            y_acc[0:64, :], rbs[:])

                # ---- out-proj for this chunk + ReduceScatter ---------
                if rs1:
                    slot1 = {1: 0, 2: 1, 3: 2, 0: 3}[qc]
                elif rs_merge:
                    # chunks are RS'd in pairs: att order (1,2,3,0) ->
                    # pair 0 = chunks {1,2}, pair 1 = chunks {3,0}.
                    # rows interleaved per 128-row j-tile so rank r's
                    # scatter block [256r:256r+256] = [chunk_a j=r;
                    # chunk_b j=r].
                    pair, slot = {1: (0, 0), 2: (0, 1),
                                  3: (1, 0), 0: (1, 1)}[qc]
                for j in range(4):
                    m = 4 * qc + j
                    for nn in range(C // 512):
                        acc = ps_acc.tile([128, 512], F32, name="acc", tag="acc")
                        for k in range(n_yt):
                            mm(acc[:], yT_sb[k][:, 128 * m:128 * (m + 1)],
                               wout_sb[k][:, 512 * nn:512 * (nn + 1)],
                               start=(k == 0), stop=(k == n_yt - 1))
                        po_sb = o_pool.tile([128, 512], BF16, name="po", tag="po")
                        nc.vector.tensor_add(po_sb[:], acc[:],
                                             bb_sb[:, 512 * nn:512 * (nn + 1)])
                        if rs1:
                            dst = rs_in1[512 * j + 128 * slot1:
                                         512 * j + 128 * slot1 + 128,
                                         512 * nn:512 * (nn + 1)]
                        elif rs_merge:
                            dst = rs_in2[pair][256 * j + 128 * slot:
                                               256 * j + 128 * slot + 128,
                                               512 * nn:512 * (nn + 1)]
                        else:
                            dst = rs_in[qc][128 * j:128 * (j + 1),
                                            512 * nn:512 * (nn + 1)]
                        nc.sync.dma_start(dst, po_sb[:])
                if rs1:
                    if slot1 == 3:   # last chunk: run the single RS
                        nc.gpsimd.collective_compute(
                            "ReduceScatter", mybir.AluOpType.add,
                            replica_groups=cfg["groups"],
                            ins=[rs_in1[:].opt()], outs=[rs_out1[:].opt()])
                        for sl, qx in ((0, 1), (1, 2), (2, 3), (3, 0)):
                            nc.gpsimd.dma_start(
                                out[rw * qx:rw * (qx + 1), :]
                                .rearrange("p f -> () (p f)"),
                                rs_out1[128 * sl:128 * sl + rw, :]
                                .rearrange("p f -> () (p f)"))
                    continue
                if rs_merge:
                    if slot == 1:   # second chunk of the pair: run the RS
                        nc.gpsimd.collective_compute(
                            "ReduceScatter", mybir.AluOpType.add,
                            replica_groups=cfg["groups"],
                            ins=[rs_in2[pair][:].opt()],
                            outs=[rs_out2[pair][:].opt()])
                        qa, qb = ((1, 2), (3, 0))[pair]
                        for half, qx in ((0, qa), (1, qb)):
                            nc.gpsimd.dma_start(
                                out[rw * qx:rw * (qx + 1), :]
                                .rearrange("p f -> () (p f)"),
                                rs_out2[pair][128 * half:128 * half + rw, :]
                                .rearrange("p f -> () (p f)"))
                    continue
                if no_rs:
                    nc.sync.dma_start(rs_out[qc][:], rs_in[qc][0:rw, :])
                else:
                    nc.gpsimd.collective_compute(
                        "ReduceScatter", mybir.AluOpType.add,
                        replica_groups=cfg["groups"],
                        ins=[rs_in[qc][:].opt()], outs=[rs_out[qc][:].opt()])
                # bf16 -> f32 via a single casting SWDGE DMA
                nc.gpsimd.dma_start(
                    out[rw * qc:rw * (qc + 1), :].rearrange("p f -> () (p f)"),
                    rs_out[qc][:].rearrange("p f -> () (p f)"))
    nc.compile()
    return nc


def shard_inputs(x, w_qkv, w_out, b_out, cfg=CFG):
    import ml_dtypes
    bf16 = ml_dtypes.bfloat16
    B, T, C, H, D, tp = (cfg["B"], cfg["T"], cfg["C"], cfg["H"], cfg["D"], cfg["tp"])
    HPC = cfg["HPC"]
    x = np.asarray(x, dtype=np.float32).astype(bf16)
    w_qkv = np.asarray(w_qkv, dtype=np.float32).astype(bf16)
    w_out = np.asarray(w_out, dtype=np.float32).astype(bf16)
    b_out = np.asarray(b_out, dtype=np.float32)

    w_q, w_k, w_v = w_qkv[:, :C], w_qkv[:, C:2 * C], w_qkv[:, 2 * C:]
    kp = np.arange(128)[:, None]
    qf = np.arange(128)[None, :]
    mask = (kp <= qf).astype(bf16)
    b_bcast = np.ascontiguousarray(np.broadcast_to(b_out / tp, (128, C)))

    in_maps = []
    for c in range(cfg["n_cores"]):
        b, r = divmod(c, tp)
        heads = range(HPC * r, HPC * (r + 1))
        heads = list(heads)
        blocks = []
        for hp in range(len(heads) // 2):
            g0, g1 = heads[2 * hp], heads[2 * hp + 1]
            blocks.append(np.concatenate(
                [w_q[:, 64 * g0:64 * (g0 + 1)], w_q[:, 64 * g1:64 * (g1 + 1)]], axis=1))
            blocks.append(np.concatenate(
                [w_k[:, 64 * g0:64 * (g0 + 1)], w_k[:, 64 * g1:64 * (g1 + 1)]], axis=1))
        wqk_c = np.concatenate(blocks, axis=1)
        wv_c = np.concatenate([w_v[:, 64 * g:64 * (g + 1)] for g in heads], axis=1)
        wout_c = np.concatenate([w_out[64 * g:64 * (g + 1), :] for g in heads], axis=0)
        in_maps.append({
            "xT": np.ascontiguousarray(x[b].T),
            "w_qk": np.ascontiguousarray(wqk_c),
            "w_v": np.ascontiguousarray(wv_c),
            "w_out": np.ascontiguousarray(wout_c),
            "b_bcast": b_bcast,
            "mask": mask,
            "ones": np.ones((128, 64), dtype=bf16),
        })
    return in_maps


def assemble(results, cfg=CFG):
    B, T, C, tp, NQ = cfg["B"], cfg["T"], cfg["C"], cfg["tp"], cfg["NQ"]
    rw = 512 // tp
    out = np.empty((B, T, C), dtype=np.float32)
    for c in range(cfg["n_cores"]):
        b, r = divmod(c, tp)
        o = results[c]["out"]
        for qc in range(NQ):
            out[b, 512 * qc + rw * r:512 * qc + rw * (r + 1), :] = \
                o[rw * qc:rw * (qc + 1)]
    return out


_NC_CACHE = {}


def _get_nc(cfg_key="default", cfg=CFG):
    if cfg_key not in _NC_CACHE:
        _NC_CACHE[cfg_key] = build_nc(cfg)
    return _NC_CACHE[cfg_key]


def kernel(x, w_qkv, w_out, b_out):
    cfg = CFG
    nc = _get_nc()
    in_maps = shard_inputs(x, w_qkv, w_out, b_out, cfg)
    res = bass_utils.run_bass_kernel_spmd(
        nc, in_maps, core_ids=list(range(cfg["n_cores"])))
    return assemble(res.results, cfg)


if __name__ == "__main__":
    print("module loads ok")

